# revision 1
# baseline (speedup 1.0000x reference)
"""Trainium2 Bass kernel for Conv2dAffine8bit.

Reference computation:
    w_dq = affine-uint8 quantize-dequantize(weight)   (per-tensor scale/zp)
    out  = conv2d(x, w_dq, stride 1, pad 1) + bias    (NCHW, OIHW)
with x [32, 256, 56, 56] f32, weight [256, 256, 3, 3] f32, bias [256] f32.

Strategy (8 NeuronCores, data-parallel over batch, 4 images per core):
  * Host reproduces the quantization math bit-exactly in fp32. The integer
    weights (w_q - zero_point, in [-255, 255]) are EXACT in bf16/f32r, so the
    conv runs against integer weights and `scale` folds into the epilogue
    (conv is linear in w).
  * conv = 9 shifted matmuls (3x3 taps) over a zero-padded [58, 58] image in
    SBUF: out[co, r, :] accumulates lhsT[ci, co] @ x[ci, r+ky, kx:kx+56] into
    PSUM tiles of [128 cout, 8 rows x 56 cols = 448].
  * Two precision modes:
      - "f32r" (default): PE's 12-bit-significand fp32 mode, full bf16-rate
        for free dim >= 256. 9 taps x 2 cin-halves = 18 matmuls per PSUM
        tile; ~1e-4 relative error.
      - "hilo": x split into bf16 hi + lo (x = hi + lo keeps ~16 mantissa
        bits); 36 matmuls per PSUM tile; ~3e-6 relative error, 2x the PE time.
  * Epilogue: ScalarE Identity activation computes psum * scale + bias[co],
    then DMA to the output shard.
"""

import os

import numpy as np
import ml_dtypes

import concourse.tile as tile
from concourse import bacc, mybir
from concourse.bass_utils import run_bass_kernel_spmd

N_CORES = 8
N_IMGS = 32
IMGS_PER_CORE = N_IMGS // N_CORES
C = 256  # in and out channels
H = W = 56
HP = WP = 58  # padded
R = 8  # output rows per PSUM block
N_BLOCKS = H // R  # 7
FREE = R * W  # 448

MODE = os.environ.get("CONV_MODE", "f32r")  # "f32r" | "hilo"

_BF16 = ml_dtypes.bfloat16

_cache: dict = {}


def _build(scale: float, mode: str):
    """Build + compile the per-core Bass program. `scale` is baked as an
    immediate in the epilogue, so cache on it."""
    key = (scale, mode)
    if key in _cache:
        return _cache[key]

    nc = bacc.Bacc()
    dt = mybir.dt
    x_dt = dt.float32r if mode == "f32r" else dt.bfloat16
    w_dt = x_dt
    if mode == "f32r":
        d_xs = [nc.declare_dram_parameter(
            "x0", [IMGS_PER_CORE, C, HP, WP], x_dt, isOutput=False)]
        parts = ("x0",)
    else:
        d_xs = [
            nc.declare_dram_parameter(
                "xhi", [IMGS_PER_CORE, C, HP, WP], x_dt, isOutput=False),
            nc.declare_dram_parameter(
                "xlo", [IMGS_PER_CORE, C, HP, WP], x_dt, isOutput=False),
        ]
        parts = ("xhi", "xlo")
    # wq[ci, ci_half, tap, co_half, co] = (w_q - zp)[co_half*128+co, ci_half*128+ci, tap]
    d_wq = nc.declare_dram_parameter("wq", [128, 2, 9, 2, 128], w_dt, isOutput=False)
    d_bias = nc.declare_dram_parameter("bias", [128, 2], dt.float32, isOutput=False)
    d_y = nc.declare_dram_parameter(
        "y", [IMGS_PER_CORE, C, H, W], dt.float32, isOutput=True)

    with tile.TileContext(nc) as tc:
        with (
            tc.tile_pool(name="wpool", bufs=1) as wpool,
            tc.tile_pool(name="xpool", bufs=3) as xpool,
            tc.tile_pool(name="opool", bufs=4) as opool,
            tc.tile_pool(name="pspool", bufs=4, space="PSUM") as pspool,
        ):
            t_wq = wpool.tile([128, 2, 9, 2, 128], w_dt, tag="wq")
            nc.sync.dma_start(t_wq[:], d_wq[:])
            t_bias = wpool.tile([128, 2], dt.float32, tag="bias")
            nc.sync.dma_start(t_bias[:], d_bias[:])

            n_mm = 18 * len(parts)
            for img in range(IMGS_PER_CORE):
                # x tiles for this image: [128 ci, 58, 58] per (half, part)
                xt = {}
                for ci_half in range(2):
                    for part, src in zip(parts, d_xs):
                        t = xpool.tile([128, HP, WP], x_dt,
                                       tag=f"x_{part}{ci_half}")
                        nc.sync.dma_start(
                            t[:], src[img, ci_half * 128:(ci_half + 1) * 128])
                        xt[(ci_half, part)] = t

                for co_half in range(2):
                    for blk in range(N_BLOCKS):
                        r0 = blk * R
                        ps = pspool.tile([128, FREE], dt.float32, tag="ps")
                        i_mm = 0
                        for ky in (0, -1, 1):
                            for kx in (-1, 0, 1):
                                tap = (ky + 1) * 3 + (kx + 1)
                                for ci_half in range(2):
                                    lhsT = t_wq[:, ci_half, tap, co_half, :]
                                    for part in parts:
                                        rhs = xt[(ci_half, part)][
                                            :, r0 + ky + 1: r0 + ky + 1 + R,
                                            kx + 1: kx + 1 + W]
                                        nc.tensor.matmul(
                                            ps[:], lhsT, rhs,
                                            start=(i_mm == 0),
                                            stop=(i_mm == n_mm - 1))
                                        i_mm += 1
                        ob = opool.tile([128, FREE], dt.float32, tag="ob")
                        nc.scalar.activation(
                            ob[:], ps[:], mybir.ActivationFunctionType.Identity,
                            bias=t_bias[:, co_half:co_half + 1], scale=float(scale))
                        nc.scalar.dma_start(
                            d_y[img, co_half * 128:(co_half + 1) * 128,
                                r0:r0 + R, :],
                            ob[:].rearrange("p (r c) -> p r c", c=W))

    nc.compile()
    _cache[key] = nc
    return nc


def _quantize_weight(weight: np.ndarray):
    """Bit-exact fp32 replica of the reference affine-uint8 quantization.
    Returns (w_int, scale): w_int = w_q - zero_point (integers in [-255, 255],
    exact in bf16) and the per-tensor fp32 scale."""
    w = np.asarray(weight, dtype=np.float32)
    min_val = np.min(w)
    max_val = np.max(w)
    scale = np.float32(np.float32(max_val - min_val) / np.float32(255.0))
    zp = np.round(np.clip(np.float32(255.0) - np.float32(max_val / scale),
                          np.float32(0.0), np.float32(255.0)))
    w_q = np.round(np.clip(zp + w / scale, np.float32(0.0), np.float32(255.0)))
    w_int = (w_q - zp).astype(np.float32)
    return w_int, scale


def kernel(x, weight, bias):
    x = np.asarray(x, dtype=np.float32)
    weight = np.asarray(weight, dtype=np.float32)
    bias = np.asarray(bias, dtype=np.float32)

    w_int, scale = _quantize_weight(weight)

    # lhsT layout [ci, ci_half, tap, co_half, co]
    w_r = w_int.reshape(2, 128, 2, 128, 9)  # [co_half, co, ci_half, ci, tap]
    wq_host = np.ascontiguousarray(np.transpose(w_r, (3, 2, 4, 0, 1)))
    bias_host = np.ascontiguousarray(bias.reshape(2, 128).T)  # [128, 2]

    # pad to [N, C, 58, 58]
    xp = np.zeros((N_IMGS, C, HP, WP), dtype=np.float32)
    xp[:, :, 1:1 + H, 1:1 + W] = x

    if MODE == "f32r":
        x_parts = {"x0": xp}  # raw f32 bits; PE rounds on ingest
        wq_host = wq_host.astype(np.float32)
    else:
        x_hi = xp.astype(_BF16)
        x_lo = (xp - x_hi.astype(np.float32)).astype(_BF16)
        x_parts = {"xhi": x_hi, "xlo": x_lo}
        wq_host = wq_host.astype(_BF16)

    nc = _build(float(scale), MODE)
    in_maps = []
    for c in range(N_CORES):
        sl = slice(c * IMGS_PER_CORE, (c + 1) * IMGS_PER_CORE)
        m = {name: arr[sl] for name, arr in x_parts.items()}
        m["wq"] = wq_host
        m["bias"] = bias_host
        in_maps.append(m)
    res = run_bass_kernel_spmd(nc, in_maps, list(range(N_CORES)))
    return np.concatenate([res.results[c]["y"] for c in range(N_CORES)], axis=0)



# revision 5
# speedup vs baseline: 25080.7242x; 25080.7242x over previous
"""Trainium2 Bass kernel for Conv2dAffine8bit.

Reference computation:
    w_dq = affine-uint8 quantize-dequantize(weight)   (per-tensor scale/zp)
    out  = conv2d(x, w_dq, stride 1, pad 1) + bias    (NCHW, OIHW)
with x [32, 256, 56, 56] f32, weight [256, 256, 3, 3] f32, bias [256] f32.

Strategy (8 NeuronCores, data-parallel over batch, 4 images per core):
  * Host reproduces the quantization math bit-exactly in fp32. The integer
    weights (w_q - zero_point, in [-255, 255]) are EXACT in bf16/f32r, so the
    conv runs against integer weights and `scale` folds into the epilogue
    (conv is linear in w).
  * conv = 9 shifted matmuls (3x3 taps) over a zero-padded [58, 58] image in
    SBUF: out[co, r, :] accumulates lhsT[ci, co] @ x[ci, r+ky, kx:kx+56] into
    PSUM tiles of [128 cout, 8 rows x 56 cols = 448].
  * Two precision modes:
      - "f32r" (default): PE's 12-bit-significand fp32 mode, full bf16-rate
        for free dim >= 256. 9 taps x 2 cin-halves = 18 matmuls per PSUM
        tile; ~1e-4 relative error.
      - "hilo": x split into bf16 hi + lo (x = hi + lo keeps ~16 mantissa
        bits); 36 matmuls per PSUM tile; ~3e-6 relative error, 2x the PE time.
  * Epilogue: ScalarE Identity activation computes psum * scale + bias[co],
    then DMA to the output shard.
"""

import os

import numpy as np
import ml_dtypes

import concourse.tile as tile
from concourse import bacc, mybir
from concourse.bass_utils import run_bass_kernel_spmd

N_CORES = 8
N_IMGS = 32
IMGS_PER_CORE = N_IMGS // N_CORES
C = 256  # in and out channels
H = W = 56
HP = WP = 58  # padded
R = 8  # output rows per PSUM block
N_BLOCKS = H // R  # 7
FREE = R * W  # 448

MODE = os.environ.get("CONV_MODE", "f32r")  # "f32r" | "hilo"

_BF16 = ml_dtypes.bfloat16

_cache: dict = {}


def _build(scale: float, mode: str, reps: int = 1, unroll: int = 1):
    """Build + compile the per-core Bass program. `scale` is baked as an
    immediate in the epilogue, so cache on it. reps/unroll (timing rig only)
    wrap the body in a For_i hardware loop with `unroll` copies inside."""
    key = (scale, mode, reps, unroll)
    if key in _cache:
        return _cache[key]

    nc = bacc.Bacc()
    dt = mybir.dt
    x_dt = dt.float32r if mode == "f32r" else dt.bfloat16
    w_dt = x_dt
    if mode == "f32r":
        d_xs = [nc.declare_dram_parameter(
            "x0", [IMGS_PER_CORE, C, HP, WP], x_dt, isOutput=False)]
        parts = ("x0",)
    else:
        d_xs = [
            nc.declare_dram_parameter(
                "xhi", [IMGS_PER_CORE, C, HP, WP], x_dt, isOutput=False),
            nc.declare_dram_parameter(
                "xlo", [IMGS_PER_CORE, C, HP, WP], x_dt, isOutput=False),
        ]
        parts = ("xhi", "xlo")
    # wq[ci, ci_half, tap, co_half, co] = (w_q - zp)[co_half*128+co, ci_half*128+ci, tap]
    d_wq = nc.declare_dram_parameter("wq", [128, 2, 9, 2, 128], w_dt, isOutput=False)
    d_bias = nc.declare_dram_parameter("bias", [128, 2], dt.float32, isOutput=False)
    d_y = nc.declare_dram_parameter(
        "y", [IMGS_PER_CORE, C, H, W], dt.float32, isOutput=True)

    with tile.TileContext(nc) as tc:
        with (
            tc.tile_pool(name="wpool", bufs=1) as wpool,
            tc.tile_pool(name="xpool", bufs=3) as xpool,
            tc.tile_pool(name="opool", bufs=4) as opool,
            tc.tile_pool(name="pspool", bufs=4, space="PSUM") as pspool,
        ):
            t_wq = wpool.tile([128, 2, 9, 2, 128], w_dt, tag="wq")
            nc.sync.dma_start(t_wq[:], d_wq[:])
            t_bias = wpool.tile([128, 2], dt.float32, tag="bias")
            nc.sync.dma_start(t_bias[:], d_bias[:])

            import contextlib
            loop_cm = (tc.For_i(0, reps) if (reps > 1 or unroll > 1)
                       else contextlib.nullcontext())
            with loop_cm:
              for _u in range(unroll):
                _emit_body(nc, tc, xpool, opool, pspool, t_wq, t_bias,
                           d_xs, d_y, parts, x_dt, scale)

    nc.compile()
    _cache[key] = nc
    return nc


def _emit_body(nc, tc, xpool, opool, pspool, t_wq, t_bias, d_xs, d_y,
               parts, x_dt, scale):
            dt = mybir.dt
            n_mm = 18 * len(parts)
            for img in range(IMGS_PER_CORE):
                # x tiles for this image: [128 ci, 58, 58] per (half, part)
                xt = {}
                for ci_half in range(2):
                    for part, src in zip(parts, d_xs):
                        t = xpool.tile([128, HP, WP], x_dt,
                                       tag=f"x_{part}{ci_half}")
                        nc.sync.dma_start(
                            t[:], src[img, ci_half * 128:(ci_half + 1) * 128])
                        xt[(ci_half, part)] = t

                for co_half in range(2):
                    for blk in range(N_BLOCKS):
                        r0 = blk * R
                        ps = pspool.tile([128, FREE], dt.float32, tag="ps")
                        i_mm = 0
                        for ky in (0, -1, 1):
                            for kx in (-1, 0, 1):
                                tap = (ky + 1) * 3 + (kx + 1)
                                for ci_half in range(2):
                                    lhsT = t_wq[:, ci_half, tap, co_half, :]
                                    for part in parts:
                                        rhs = xt[(ci_half, part)][
                                            :, r0 + ky + 1: r0 + ky + 1 + R,
                                            kx + 1: kx + 1 + W]
                                        nc.tensor.matmul(
                                            ps[:], lhsT, rhs,
                                            start=(i_mm == 0),
                                            stop=(i_mm == n_mm - 1))
                                        i_mm += 1
                        ob = opool.tile([128, FREE], dt.float32, tag="ob")
                        nc.scalar.activation(
                            ob[:], ps[:], mybir.ActivationFunctionType.Identity,
                            bias=t_bias[:, co_half:co_half + 1], scale=float(scale))
                        nc.scalar.dma_start(
                            d_y[img, co_half * 128:(co_half + 1) * 128,
                                r0:r0 + R, :],
                            ob[:].rearrange("p (r c) -> p r c", c=W))


def _quantize_weight(weight: np.ndarray):
    """Bit-exact fp32 replica of the reference affine-uint8 quantization.
    Returns (w_int, scale): w_int = w_q - zero_point (integers in [-255, 255],
    exact in bf16) and the per-tensor fp32 scale."""
    w = np.asarray(weight, dtype=np.float32)
    min_val = np.min(w)
    max_val = np.max(w)
    scale = np.float32(np.float32(max_val - min_val) / np.float32(255.0))
    zp = np.round(np.clip(np.float32(255.0) - np.float32(max_val / scale),
                          np.float32(0.0), np.float32(255.0)))
    w_q = np.round(np.clip(zp + w / scale, np.float32(0.0), np.float32(255.0)))
    w_int = (w_q - zp).astype(np.float32)
    return w_int, scale


def _make_in_maps(x, weight, bias):
    x = np.asarray(x, dtype=np.float32)
    weight = np.asarray(weight, dtype=np.float32)
    bias = np.asarray(bias, dtype=np.float32)

    w_int, scale = _quantize_weight(weight)

    # lhsT layout [ci, ci_half, tap, co_half, co]
    w_r = w_int.reshape(2, 128, 2, 128, 9)  # [co_half, co, ci_half, ci, tap]
    wq_host = np.ascontiguousarray(np.transpose(w_r, (3, 2, 4, 0, 1)))
    bias_host = np.ascontiguousarray(bias.reshape(2, 128).T)  # [128, 2]

    # pad to [N, C, 58, 58]
    xp = np.zeros((N_IMGS, C, HP, WP), dtype=np.float32)
    xp[:, :, 1:1 + H, 1:1 + W] = x

    if MODE == "f32r":
        x_parts = {"x0": xp}  # raw f32 bits; PE rounds on ingest
        wq_host = wq_host.astype(np.float32)
    else:
        x_hi = xp.astype(_BF16)
        x_lo = (xp - x_hi.astype(np.float32)).astype(_BF16)
        x_parts = {"xhi": x_hi, "xlo": x_lo}
        wq_host = wq_host.astype(_BF16)

    in_maps = []
    for c in range(N_CORES):
        sl = slice(c * IMGS_PER_CORE, (c + 1) * IMGS_PER_CORE)
        m = {name: arr[sl] for name, arr in x_parts.items()}
        m["wq"] = wq_host
        m["bias"] = bias_host
        in_maps.append(m)
    return in_maps


def kernel(x, weight, bias):
    weight = np.asarray(weight, dtype=np.float32)
    _, scale = _quantize_weight(weight)
    in_maps = _make_in_maps(x, weight, bias)
    nc = _build(float(scale), MODE)
    res = run_bass_kernel_spmd(nc, in_maps, list(range(N_CORES)))
    return np.concatenate([res.results[c]["y"] for c in range(N_CORES)], axis=0)



# revision 8
# speedup vs baseline: 28821.9632x; 1.1492x over previous
"""Trainium2 Bass kernel for Conv2dAffine8bit.

Reference computation:
    w_dq = affine-uint8 quantize-dequantize(weight)   (per-tensor scale/zp)
    out  = conv2d(x, w_dq, stride 1, pad 1) + bias    (NCHW, OIHW)
with x [32, 256, 56, 56] f32, weight [256, 256, 3, 3] f32, bias [256] f32.

Strategy (8 NeuronCores, data-parallel over batch, 4 images per core):
  * Host reproduces the quantization math bit-exactly in fp32. The integer
    weights (w_q - zero_point, in [-255, 255]) are EXACT in bf16/f32r, so the
    conv runs against integer weights and `scale` folds into the epilogue
    (conv is linear in w).
  * conv = 9 shifted matmuls (3x3 taps) over a zero-padded [58, 58] image in
    SBUF: out[co, r, :] accumulates lhsT[ci, co] @ x[ci, r+ky, kx:kx+56] into
    PSUM tiles of [128 cout, 8 rows x 56 cols = 448].
  * Two precision modes:
      - "f32r" (default): PE's 12-bit-significand fp32 mode, full bf16-rate
        for free dim >= 256. 9 taps x 2 cin-halves = 18 matmuls per PSUM
        tile; ~1e-4 relative error.
      - "hilo": x split into bf16 hi + lo (x = hi + lo keeps ~16 mantissa
        bits); 36 matmuls per PSUM tile; ~3e-6 relative error, 2x the PE time.
  * Epilogue: ScalarE Identity activation computes psum * scale + bias[co],
    then DMA to the output shard.
"""

import os

import numpy as np
import ml_dtypes

import concourse.tile as tile
from concourse import bacc, mybir
from concourse.bass_utils import run_bass_kernel_spmd

N_CORES = 8
N_IMGS = 32
IMGS_PER_CORE = N_IMGS // N_CORES
C = 256  # in and out channels
H = W = 56
HP = WP = 58  # padded
R = 8  # output rows per PSUM block
N_BLOCKS = H // R  # 7
FREE = R * W  # 448

MODE = os.environ.get("CONV_MODE", "bf16")  # "bf16" | "f32r" | "hilo"

_BF16 = ml_dtypes.bfloat16

_cache: dict = {}


def _build(scale: float, mode: str, reps: int = 1, unroll: int = 1):
    """Build + compile the per-core Bass program. `scale` is baked as an
    immediate in the epilogue, so cache on it. reps/unroll (timing rig only)
    wrap the body in a For_i hardware loop with `unroll` copies inside."""
    key = (scale, mode, reps, unroll)
    if key in _cache:
        return _cache[key]

    nc = bacc.Bacc()
    dt = mybir.dt
    x_dt = dt.float32r if mode == "f32r" else dt.bfloat16
    w_dt = x_dt
    if mode in ("f32r", "bf16"):
        d_xs = [nc.declare_dram_parameter(
            "x0", [IMGS_PER_CORE, C, HP, WP], x_dt, isOutput=False)]
        parts = ("x0",)
    else:
        d_xs = [
            nc.declare_dram_parameter(
                "xhi", [IMGS_PER_CORE, C, HP, WP], x_dt, isOutput=False),
            nc.declare_dram_parameter(
                "xlo", [IMGS_PER_CORE, C, HP, WP], x_dt, isOutput=False),
        ]
        parts = ("xhi", "xlo")
    # wq[ci, ci_half, tap, co_half, co] = (w_q - zp)[co_half*128+co, ci_half*128+ci, tap]
    d_wq = nc.declare_dram_parameter("wq", [128, 2, 9, 2, 128], w_dt, isOutput=False)
    d_bias = nc.declare_dram_parameter("bias", [128, 2], dt.float32, isOutput=False)
    d_y = nc.declare_dram_parameter(
        "y", [IMGS_PER_CORE, C, H, W], dt.float32, isOutput=True)

    with tile.TileContext(nc) as tc:
        with (
            tc.tile_pool(name="wpool", bufs=1) as wpool,
            tc.tile_pool(name="xpool", bufs=3) as xpool,
            tc.tile_pool(name="opool", bufs=4) as opool,
            tc.tile_pool(name="pspool", bufs=4, space="PSUM") as pspool,
        ):
            t_wq = wpool.tile([128, 2, 9, 2, 128], w_dt, tag="wq")
            nc.sync.dma_start(t_wq[:], d_wq[:])
            t_bias = wpool.tile([128, 2], dt.float32, tag="bias")
            nc.sync.dma_start(t_bias[:], d_bias[:])

            import contextlib
            loop_cm = (tc.For_i(0, reps) if (reps > 1 or unroll > 1)
                       else contextlib.nullcontext())
            with loop_cm:
              for _u in range(unroll):
                _emit_body(nc, tc, xpool, opool, pspool, t_wq, t_bias,
                           d_xs, d_y, parts, x_dt, scale)

    nc.compile()
    _cache[key] = nc
    return nc


def _emit_body(nc, tc, xpool, opool, pspool, t_wq, t_bias, d_xs, d_y,
               parts, x_dt, scale):
            dt = mybir.dt
            n_mm = 18 * len(parts)
            for img in range(IMGS_PER_CORE):
                # x tiles for this image: [128 ci, 58, 58] per (half, part)
                xt = {}
                for ci_half in range(2):
                    for part, src in zip(parts, d_xs):
                        t = xpool.tile([128, HP, WP], x_dt,
                                       tag=f"x_{part}{ci_half}")
                        nc.sync.dma_start(
                            t[:], src[img, ci_half * 128:(ci_half + 1) * 128])
                        xt[(ci_half, part)] = t

                for co_half in range(2):
                    for blk in range(N_BLOCKS):
                        r0 = blk * R
                        ps = pspool.tile([128, FREE], dt.float32, tag="ps")
                        i_mm = 0
                        for ky in (0, -1, 1):
                            for kx in (-1, 0, 1):
                                tap = (ky + 1) * 3 + (kx + 1)
                                for ci_half in range(2):
                                    lhsT = t_wq[:, ci_half, tap, co_half, :]
                                    for part in parts:
                                        rhs = xt[(ci_half, part)][
                                            :, r0 + ky + 1: r0 + ky + 1 + R,
                                            kx + 1: kx + 1 + W]
                                        nc.tensor.matmul(
                                            ps[:], lhsT, rhs,
                                            start=(i_mm == 0),
                                            stop=(i_mm == n_mm - 1))
                                        i_mm += 1
                        ob = opool.tile([128, FREE], dt.float32, tag="ob")
                        nc.scalar.activation(
                            ob[:], ps[:], mybir.ActivationFunctionType.Identity,
                            bias=t_bias[:, co_half:co_half + 1], scale=float(scale))
                        nc.scalar.dma_start(
                            d_y[img, co_half * 128:(co_half + 1) * 128,
                                r0:r0 + R, :],
                            ob[:].rearrange("p (r c) -> p r c", c=W))


def _quantize_weight(weight: np.ndarray):
    """Bit-exact fp32 replica of the reference affine-uint8 quantization.
    Returns (w_int, scale): w_int = w_q - zero_point (integers in [-255, 255],
    exact in bf16) and the per-tensor fp32 scale."""
    w = np.asarray(weight, dtype=np.float32)
    min_val = np.min(w)
    max_val = np.max(w)
    scale = np.float32(np.float32(max_val - min_val) / np.float32(255.0))
    zp = np.round(np.clip(np.float32(255.0) - np.float32(max_val / scale),
                          np.float32(0.0), np.float32(255.0)))
    w_q = np.round(np.clip(zp + w / scale, np.float32(0.0), np.float32(255.0)))
    w_int = (w_q - zp).astype(np.float32)
    return w_int, scale


def _make_in_maps(x, weight, bias):
    x = np.asarray(x, dtype=np.float32)
    weight = np.asarray(weight, dtype=np.float32)
    bias = np.asarray(bias, dtype=np.float32)

    w_int, scale = _quantize_weight(weight)

    # lhsT layout [ci, ci_half, tap, co_half, co]
    w_r = w_int.reshape(2, 128, 2, 128, 9)  # [co_half, co, ci_half, ci, tap]
    wq_host = np.ascontiguousarray(np.transpose(w_r, (3, 2, 4, 0, 1)))
    bias_host = np.ascontiguousarray(bias.reshape(2, 128).T)  # [128, 2]

    # pad to [N, C, 58, 58]
    xp = np.zeros((N_IMGS, C, HP, WP), dtype=np.float32)
    xp[:, :, 1:1 + H, 1:1 + W] = x

    if MODE == "f32r":
        x_parts = {"x0": xp}  # raw f32 bits; PE rounds on ingest
        wq_host = wq_host.astype(np.float32)
    elif MODE == "bf16":
        # single bf16 part: w_int (integers in [-255, 255]) is exact in
        # bf16; only x rounds, giving ~1e-3 relative output error
        x_parts = {"x0": xp.astype(_BF16)}
        wq_host = wq_host.astype(_BF16)
    else:
        x_hi = xp.astype(_BF16)
        x_lo = (xp - x_hi.astype(np.float32)).astype(_BF16)
        x_parts = {"xhi": x_hi, "xlo": x_lo}
        wq_host = wq_host.astype(_BF16)

    in_maps = []
    for c in range(N_CORES):
        sl = slice(c * IMGS_PER_CORE, (c + 1) * IMGS_PER_CORE)
        m = {name: arr[sl] for name, arr in x_parts.items()}
        m["wq"] = wq_host
        m["bias"] = bias_host
        in_maps.append(m)
    return in_maps


def kernel(x, weight, bias):
    weight = np.asarray(weight, dtype=np.float32)
    _, scale = _quantize_weight(weight)
    in_maps = _make_in_maps(x, weight, bias)
    nc = _build(float(scale), MODE)
    res = run_bass_kernel_spmd(nc, in_maps, list(range(N_CORES)))
    return np.concatenate([res.results[c]["y"] for c in range(N_CORES)], axis=0)



# revision 12
# speedup vs baseline: 42885.8281x; 1.4880x over previous
"""Trainium2 Bass kernel for Conv2dAffine8bit.

Reference computation:
    w_dq = affine-uint8 quantize-dequantize(weight)   (per-tensor scale/zp)
    out  = conv2d(x, w_dq, stride 1, pad 1) + bias    (NCHW, OIHW)
with x [32, 256, 56, 56] f32, weight [256, 256, 3, 3] f32, bias [256] f32.

Strategy (8 NeuronCores, data-parallel over batch, 4 images per core):
  * Host reproduces the quantization math bit-exactly in fp32. The integer
    weights (w_q - zero_point) are EXACT in bf16, so the conv runs against
    integer weights and `scale` folds into the epilogue (conv is linear in w).
  * MODE "wino" (default): 1D Winograd F(2,3) along the width axis.
    - host: G0 = w0, G1 = (w0+w1+w2)/2, G2 = (w0-w1+w2)/2, G3 = w2 per
      kernel row ky (all exact in bf16 for this weight range).
    - device DVE: width transform of the zero-padded bf16 image
      (58 cols -> 28 tiles x 4 domains, pure add/sub).
    - PE: per output-row group, 4 domain accumulators in PSUM; each gets
      3 ky x 2 ci_half matmuls [128ci, 128co] x [128ci, 14x28].
      24 MMs of free-dim 392 per group vs 36 equivalent direct-conv MMs:
      1.5x fewer PE cycles.
    - inverse transform o_even = m0+m1+m2, o_odd = m1-m2-m3 on DVE
      (the F(2,3) 1/2 factors are pre-folded into G1, G2), interleaved
      into a [128, 14, 56] tile; ScalarE applies scale*o + bias; DMA out.
  * MODE "bf16": direct conv, 9 taps x 2 ci_half = 18 MMs per PSUM tile of
    [128 co, 8 rows x 56 cols]. ~1.5e-3 relative error.
  * MODE "f32r"/"hilo": previous direct-conv modes (fp32 ingest variants).
"""

import contextlib
import os

import numpy as np
import ml_dtypes

import concourse.tile as tile
from concourse import bacc, mybir
from concourse.bass_utils import run_bass_kernel_spmd

N_CORES = 8
N_IMGS = 32
IMGS_PER_CORE = N_IMGS // N_CORES
C = 256  # in and out channels
H = W = 56
HP = WP = 58  # padded
R = 8  # output rows per PSUM block (direct modes)
N_BLOCKS = H // R  # 7
FREE = R * W  # 448

# winograd mode dims
T = 28  # width tiles (2 output cols each)
RG = 14  # output rows per PSUM group
NG = H // RG  # 4 groups
FREE_W = RG * T  # 392

MODE = os.environ.get("CONV_MODE", "wino")  # "wino" | "bf16" | "f32r" | "hilo"

_BF16 = ml_dtypes.bfloat16

_cache: dict = {}


def _build(scale: float, mode: str, reps: int = 1, unroll: int = 1):
    """Build + compile the per-core Bass program. `scale` is baked as an
    immediate in the epilogue, so cache on it. reps/unroll (timing rig only)
    wrap the body in a For_i hardware loop with `unroll` copies inside."""
    key = (scale, mode, reps, unroll)
    if key in _cache:
        return _cache[key]
    nc = _build_wino(scale, reps, unroll) if mode == "wino" else \
        _build_direct(scale, mode, reps, unroll)
    _cache[key] = nc
    return nc


# ──────────────────────────── winograd mode ────────────────────────────


def _build_wino(scale: float, reps: int, unroll: int):
    nc = bacc.Bacc()
    dt = mybir.dt
    d_x = nc.declare_dram_parameter(
        "x0", [IMGS_PER_CORE, C, HP, WP], dt.bfloat16, isOutput=False)
    # wg[ci, ci_half, k, ky, co_half, co] = G[k][co_half*128+co, ci_half*128+ci, ky]
    d_wg = nc.declare_dram_parameter(
        "wg", [128, 2, 4, 3, 2, 128], dt.bfloat16, isOutput=False)
    d_bias = nc.declare_dram_parameter("bias", [128, 2], dt.float32,
                                       isOutput=False)
    d_y = nc.declare_dram_parameter(
        "y", [IMGS_PER_CORE, C, H, W], dt.float32, isOutput=True)

    with tile.TileContext(nc) as tc:
        with (
            tc.tile_pool(name="wpool", bufs=1) as wpool,
            tc.tile_pool(name="xpool", bufs=3) as xpool,
            tc.tile_pool(name="dpool", bufs=3) as dpool,
            tc.tile_pool(name="tpool", bufs=4) as tpool,
            tc.tile_pool(name="opool", bufs=4) as opool,
            tc.tile_pool(name="pspool", bufs=2, space="PSUM") as pspool,
        ):
            t_wg = wpool.tile([128, 2, 4, 3, 2, 128], dt.bfloat16, tag="wg")
            nc.sync.dma_start(t_wg[:], d_wg[:])
            t_bias = wpool.tile([128, 2], dt.float32, tag="bias")
            nc.sync.dma_start(t_bias[:], d_bias[:])

            loop_cm = (tc.For_i(0, reps) if (reps > 1 or unroll > 1)
                       else contextlib.nullcontext())
            with loop_cm:
                for _u in range(unroll):
                    _emit_wino_body(nc, xpool, dpool, tpool, opool, pspool,
                                    t_wg, t_bias, d_x, d_y, scale)

    nc.compile()
    return nc


def _emit_wino_body(nc, xpool, dpool, tpool, opool, pspool, t_wg, t_bias,
                    d_x, d_y, scale):
    dt = mybir.dt
    add = mybir.AluOpType.add
    sub = mybir.AluOpType.subtract
    for img in range(IMGS_PER_CORE):
        dts = {}
        for cih in range(2):
            xs = xpool.tile([128, HP, WP], dt.bfloat16, tag=f"x{cih}")
            nc.sync.dma_start(
                xs[:], d_x[img, cih * 128:(cih + 1) * 128])
            # width transform: 58 padded cols -> 4 domains x 28 tiles
            dtile = dpool.tile([128, 4, HP, T], dt.bfloat16, tag=f"d{cih}")
            e0 = xs[:, :, 0:56:2]   # x[2t]
            e1 = xs[:, :, 1:57:2]   # x[2t+1]
            e2 = xs[:, :, 2:58:2]   # x[2t+2]
            e3 = xs[:, :, 3:58:2]   # x[2t+3]
            nc.vector.tensor_tensor(dtile[:, 0], e0, e2, sub)
            nc.vector.tensor_tensor(dtile[:, 1], e1, e2, add)
            nc.vector.tensor_tensor(dtile[:, 2], e2, e1, sub)
            nc.vector.tensor_tensor(dtile[:, 3], e1, e3, sub)
            dts[cih] = dtile

        for coh in range(2):
            for grp in range(NG):
                r0 = grp * RG
                m = [pspool.tile([128, FREE_W], dt.float32, tag=f"m{k}",
                                 name=f"m{k}") for k in range(4)]
                for k in range(4):
                    i_mm = 0
                    for ky in range(3):
                        for cih in range(2):
                            lhsT = t_wg[:, cih, k, ky, coh, :]
                            rhs = dts[cih][:, k, r0 + ky: r0 + ky + RG, :]
                            nc.tensor.matmul(m[k][:], lhsT, rhs,
                                             start=(i_mm == 0),
                                             stop=(i_mm == 5))
                            i_mm += 1
                # inverse transform: o_even = m0 + (m1 + m2),
                # o_odd = (m1 - m2) - m3   (1/2 factors folded into G1, G2).
                # DVE can read only ONE PSUM operand per op, so m1 goes to
                # SBUF first (on GpSimd, which is otherwise idle).
                c1 = tpool.tile([128, FREE_W], dt.float32, tag="c1")
                nc.scalar.activation(
                    c1[:], m[1][:], mybir.ActivationFunctionType.Identity,
                    scale=1.0)
                t_s = tpool.tile([128, FREE_W], dt.float32, tag="t_s")
                t_d = tpool.tile([128, FREE_W], dt.float32, tag="t_d")
                nc.vector.tensor_tensor(t_s[:], c1[:], m[2][:], add)
                nc.vector.tensor_tensor(t_d[:], c1[:], m[2][:], sub)
                ob = opool.tile([128, RG, W], dt.float32, tag="ob")
                ov = ob[:].rearrange("p r (t two) -> p r t two", two=2)
                nc.vector.tensor_tensor(
                    ov[:, :, :, 0], m[0][:].rearrange("p (r t) -> p r t", t=T),
                    t_s[:].rearrange("p (r t) -> p r t", t=T), add)
                nc.vector.tensor_tensor(
                    ov[:, :, :, 1], t_d[:].rearrange("p (r t) -> p r t", t=T),
                    m[3][:].rearrange("p (r t) -> p r t", t=T), sub)
                of = opool.tile([128, RG, W], dt.float32, tag="of")
                nc.scalar.activation(
                    of[:].rearrange("p r c -> p (r c)"),
                    ob[:].rearrange("p r c -> p (r c)"),
                    mybir.ActivationFunctionType.Identity,
                    bias=t_bias[:, coh:coh + 1], scale=float(scale))
                nc.scalar.dma_start(
                    d_y[img, coh * 128:(coh + 1) * 128, r0:r0 + RG, :],
                    of[:])


# ──────────────────────────── direct modes ────────────────────────────


def _build_direct(scale: float, mode: str, reps: int, unroll: int):
    nc = bacc.Bacc()
    dt = mybir.dt
    x_dt = dt.float32r if mode == "f32r" else dt.bfloat16
    w_dt = x_dt
    if mode in ("f32r", "bf16"):
        d_xs = [nc.declare_dram_parameter(
            "x0", [IMGS_PER_CORE, C, HP, WP], x_dt, isOutput=False)]
        parts = ("x0",)
    else:
        d_xs = [
            nc.declare_dram_parameter(
                "xhi", [IMGS_PER_CORE, C, HP, WP], x_dt, isOutput=False),
            nc.declare_dram_parameter(
                "xlo", [IMGS_PER_CORE, C, HP, WP], x_dt, isOutput=False),
        ]
        parts = ("xhi", "xlo")
    # wq[ci, ci_half, tap, co_half, co] = (w_q - zp)[co_half*128+co, ci_half*128+ci, tap]
    d_wq = nc.declare_dram_parameter("wq", [128, 2, 9, 2, 128], w_dt, isOutput=False)
    d_bias = nc.declare_dram_parameter("bias", [128, 2], dt.float32, isOutput=False)
    d_y = nc.declare_dram_parameter(
        "y", [IMGS_PER_CORE, C, H, W], dt.float32, isOutput=True)

    with tile.TileContext(nc) as tc:
        with (
            tc.tile_pool(name="wpool", bufs=1) as wpool,
            tc.tile_pool(name="xpool", bufs=3) as xpool,
            tc.tile_pool(name="opool", bufs=4) as opool,
            tc.tile_pool(name="pspool", bufs=4, space="PSUM") as pspool,
        ):
            t_wq = wpool.tile([128, 2, 9, 2, 128], w_dt, tag="wq")
            nc.sync.dma_start(t_wq[:], d_wq[:])
            t_bias = wpool.tile([128, 2], dt.float32, tag="bias")
            nc.sync.dma_start(t_bias[:], d_bias[:])

            loop_cm = (tc.For_i(0, reps) if (reps > 1 or unroll > 1)
                       else contextlib.nullcontext())
            with loop_cm:
                for _u in range(unroll):
                    _emit_direct_body(nc, xpool, opool, pspool, t_wq, t_bias,
                                      d_xs, d_y, parts, x_dt, scale)

    nc.compile()
    return nc


def _emit_direct_body(nc, xpool, opool, pspool, t_wq, t_bias, d_xs, d_y,
                      parts, x_dt, scale):
    dt = mybir.dt
    n_mm = 18 * len(parts)
    for img in range(IMGS_PER_CORE):
        # x tiles for this image: [128 ci, 58, 58] per (half, part)
        xt = {}
        for ci_half in range(2):
            for part, src in zip(parts, d_xs):
                t = xpool.tile([128, HP, WP], x_dt,
                               tag=f"x_{part}{ci_half}")
                nc.sync.dma_start(
                    t[:], src[img, ci_half * 128:(ci_half + 1) * 128])
                xt[(ci_half, part)] = t

        for co_half in range(2):
            for blk in range(N_BLOCKS):
                r0 = blk * R
                ps = pspool.tile([128, FREE], dt.float32, tag="ps")
                i_mm = 0
                for ky in (0, -1, 1):
                    for kx in (-1, 0, 1):
                        tap = (ky + 1) * 3 + (kx + 1)
                        for ci_half in range(2):
                            lhsT = t_wq[:, ci_half, tap, co_half, :]
                            for part in parts:
                                rhs = xt[(ci_half, part)][
                                    :, r0 + ky + 1: r0 + ky + 1 + R,
                                    kx + 1: kx + 1 + W]
                                nc.tensor.matmul(
                                    ps[:], lhsT, rhs,
                                    start=(i_mm == 0),
                                    stop=(i_mm == n_mm - 1))
                                i_mm += 1
                ob = opool.tile([128, FREE], dt.float32, tag="ob")
                nc.scalar.activation(
                    ob[:], ps[:], mybir.ActivationFunctionType.Identity,
                    bias=t_bias[:, co_half:co_half + 1], scale=float(scale))
                nc.scalar.dma_start(
                    d_y[img, co_half * 128:(co_half + 1) * 128,
                        r0:r0 + R, :],
                    ob[:].rearrange("p (r c) -> p r c", c=W))


# ──────────────────────────── host side ────────────────────────────


def _quantize_weight(weight: np.ndarray):
    """Bit-exact fp32 replica of the reference affine-uint8 quantization.
    Returns (w_int, scale): w_int = w_q - zero_point (integers in [-255, 255],
    exact in bf16) and the per-tensor fp32 scale."""
    w = np.asarray(weight, dtype=np.float32)
    min_val = np.min(w)
    max_val = np.max(w)
    scale = np.float32(np.float32(max_val - min_val) / np.float32(255.0))
    zp = np.round(np.clip(np.float32(255.0) - np.float32(max_val / scale),
                          np.float32(0.0), np.float32(255.0)))
    w_q = np.round(np.clip(zp + w / scale, np.float32(0.0), np.float32(255.0)))
    w_int = (w_q - zp).astype(np.float32)
    return w_int, scale


def _make_in_maps(x, weight, bias):
    x = np.asarray(x, dtype=np.float32)
    weight = np.asarray(weight, dtype=np.float32)
    bias = np.asarray(bias, dtype=np.float32)

    w_int, scale = _quantize_weight(weight)
    bias_host = np.ascontiguousarray(bias.reshape(2, 128).T)  # [128, 2]

    # pad to [N, C, 58, 58]
    xp = np.zeros((N_IMGS, C, HP, WP), dtype=np.float32)
    xp[:, :, 1:1 + H, 1:1 + W] = x

    if MODE == "wino":
        # G transform per kernel row, 1/2 of the inverse transform folded in
        w0, w1, w2 = w_int[..., 0], w_int[..., 1], w_int[..., 2]  # [co,ci,ky]
        G = np.stack([w0, (w0 + w1 + w2) / 2, (w0 - w1 + w2) / 2, w2],
                     axis=0)  # [4, co, ci, ky]
        # -> [ci, ci_half, k, ky, co_half, co]
        Gr = G.reshape(4, 2, 128, 2, 128, 3)  # [k, coh, co, cih, ci, ky]
        wg_host = np.ascontiguousarray(
            np.transpose(Gr, (4, 3, 0, 5, 1, 2))).astype(_BF16)
        x_parts = {"x0": xp.astype(_BF16)}
        extra = {"wg": wg_host}
    else:
        # lhsT layout [ci, ci_half, tap, co_half, co]
        w_r = w_int.reshape(2, 128, 2, 128, 9)  # [coh, co, cih, ci, tap]
        wq_host = np.ascontiguousarray(np.transpose(w_r, (3, 2, 4, 0, 1)))
        if MODE == "f32r":
            x_parts = {"x0": xp}  # raw f32 bits; PE rounds on ingest
            extra = {"wq": wq_host.astype(np.float32)}
        elif MODE == "bf16":
            # single bf16 part: w_int is exact in bf16; only x rounds,
            # giving ~1.5e-3 relative output error
            x_parts = {"x0": xp.astype(_BF16)}
            extra = {"wq": wq_host.astype(_BF16)}
        else:
            x_hi = xp.astype(_BF16)
            x_lo = (xp - x_hi.astype(np.float32)).astype(_BF16)
            x_parts = {"xhi": x_hi, "xlo": x_lo}
            extra = {"wq": wq_host.astype(_BF16)}

    in_maps = []
    for c in range(N_CORES):
        sl = slice(c * IMGS_PER_CORE, (c + 1) * IMGS_PER_CORE)
        m = {name: arr[sl] for name, arr in x_parts.items()}
        m.update(extra)
        m["bias"] = bias_host
        in_maps.append(m)
    return in_maps


def kernel(x, weight, bias):
    weight = np.asarray(weight, dtype=np.float32)
    _, scale = _quantize_weight(weight)
    in_maps = _make_in_maps(x, weight, bias)
    nc = _build(float(scale), MODE)
    res = run_bass_kernel_spmd(nc, in_maps, list(range(N_CORES)))
    return np.concatenate([res.results[c]["y"] for c in range(N_CORES)], axis=0)


# revision 13
# speedup vs baseline: 42905.9554x; 1.0005x over previous
"""Trainium2 Bass kernel for Conv2dAffine8bit.

Reference computation:
    w_dq = affine-uint8 quantize-dequantize(weight)   (per-tensor scale/zp)
    out  = conv2d(x, w_dq, stride 1, pad 1) + bias    (NCHW, OIHW)
with x [32, 256, 56, 56] f32, weight [256, 256, 3, 3] f32, bias [256] f32.

Strategy (8 NeuronCores, data-parallel over batch, 4 images per core):
  * Host reproduces the quantization math bit-exactly in fp32. The integer
    weights (w_q - zero_point) are EXACT in bf16, so the conv runs against
    integer weights and `scale` folds into the epilogue (conv is linear in w).
  * MODE "wino" (default): 1D Winograd F(2,3) along the width axis.
    - host: G0 = w0, G1 = (w0+w1+w2)/2, G2 = (w0-w1+w2)/2, G3 = w2 per
      kernel row ky (all exact in bf16 for this weight range).
    - device DVE: width transform of the zero-padded bf16 image
      (58 cols -> 28 tiles x 4 domains, pure add/sub).
    - PE: per output-row group, 4 domain accumulators in PSUM; each gets
      3 ky x 2 ci_half matmuls [128ci, 128co] x [128ci, 14x28].
      24 MMs of free-dim 392 per group vs 36 equivalent direct-conv MMs:
      1.5x fewer PE cycles.
    - inverse transform o_even = m0+m1+m2, o_odd = m1-m2-m3 on DVE
      (the F(2,3) 1/2 factors are pre-folded into G1, G2), interleaved
      into a [128, 14, 56] tile; ScalarE applies scale*o + bias; DMA out.
  * MODE "bf16": direct conv, 9 taps x 2 ci_half = 18 MMs per PSUM tile of
    [128 co, 8 rows x 56 cols]. ~1.5e-3 relative error.
  * MODE "f32r"/"hilo": previous direct-conv modes (fp32 ingest variants).
"""

import contextlib
import os

import numpy as np
import ml_dtypes

import concourse.tile as tile
from concourse import bacc, mybir
from concourse.bass_utils import run_bass_kernel_spmd

N_CORES = 8
N_IMGS = 32
IMGS_PER_CORE = N_IMGS // N_CORES
C = 256  # in and out channels
H = W = 56
HP = WP = 58  # padded
R = 8  # output rows per PSUM block (direct modes)
N_BLOCKS = H // R  # 7
FREE = R * W  # 448

# winograd mode dims
T = 28  # width tiles (2 output cols each)
RG = 14  # output rows per PSUM group
NG = H // RG  # 4 groups
FREE_W = RG * T  # 392

MODE = os.environ.get("CONV_MODE", "wino")  # "wino" | "bf16" | "f32r" | "hilo"

_BF16 = ml_dtypes.bfloat16

_cache: dict = {}


def _build(scale: float, mode: str, reps: int = 1, unroll: int = 1):
    """Build + compile the per-core Bass program. `scale` is baked as an
    immediate in the epilogue, so cache on it. reps/unroll (timing rig only)
    wrap the body in a For_i hardware loop with `unroll` copies inside."""
    key = (scale, mode, reps, unroll)
    if key in _cache:
        return _cache[key]
    nc = _build_wino(scale, reps, unroll) if mode == "wino" else \
        _build_direct(scale, mode, reps, unroll)
    _cache[key] = nc
    return nc


# ──────────────────────────── winograd mode ────────────────────────────


def _build_wino(scale: float, reps: int, unroll: int, xbufs: int = 3):
    nc = bacc.Bacc()
    dt = mybir.dt
    d_x = nc.declare_dram_parameter(
        "x0", [IMGS_PER_CORE, C, HP, WP], dt.bfloat16, isOutput=False)
    # wg[ci, ci_half, k, ky, co_half, co] = G[k][co_half*128+co, ci_half*128+ci, ky]
    d_wg = nc.declare_dram_parameter(
        "wg", [128, 2, 4, 3, 2, 128], dt.bfloat16, isOutput=False)
    d_bias = nc.declare_dram_parameter("bias", [128, 2], dt.float32,
                                       isOutput=False)
    d_y = nc.declare_dram_parameter(
        "y", [IMGS_PER_CORE, C, H, W], dt.float32, isOutput=True)

    with tile.TileContext(nc) as tc:
        with (
            tc.tile_pool(name="wpool", bufs=1) as wpool,
            tc.tile_pool(name="xpool", bufs=xbufs) as xpool,
            tc.tile_pool(name="dpool", bufs=3) as dpool,
            tc.tile_pool(name="tpool", bufs=4) as tpool,
            tc.tile_pool(name="opool", bufs=4) as opool,
            tc.tile_pool(name="pspool", bufs=2, space="PSUM") as pspool,
        ):
            t_wg = wpool.tile([128, 2, 4, 3, 2, 128], dt.bfloat16, tag="wg")
            nc.sync.dma_start(t_wg[:], d_wg[:])
            t_bias = wpool.tile([128, 2], dt.float32, tag="bias")
            nc.sync.dma_start(t_bias[:], d_bias[:])

            loop_cm = (tc.For_i(0, reps) if (reps > 1 or unroll > 1)
                       else contextlib.nullcontext())
            with loop_cm:
                for _u in range(unroll):
                    _emit_wino_body(nc, xpool, dpool, tpool, opool, pspool,
                                    t_wg, t_bias, d_x, d_y, scale)

    nc.compile()
    return nc


def _emit_wino_body(nc, xpool, dpool, tpool, opool, pspool, t_wg, t_bias,
                    d_x, d_y, scale):
    dt = mybir.dt
    add = mybir.AluOpType.add
    sub = mybir.AluOpType.subtract
    for img in range(IMGS_PER_CORE):
        dts = {}
        for cih in range(2):
            xs = xpool.tile([128, HP, WP], dt.bfloat16, tag=f"x{cih}")
            nc.sync.dma_start(
                xs[:], d_x[img, cih * 128:(cih + 1) * 128])
            # width transform: 58 padded cols -> 4 domains x 28 tiles
            dtile = dpool.tile([128, 4, HP, T], dt.bfloat16, tag=f"d{cih}")
            e0 = xs[:, :, 0:56:2]   # x[2t]
            e1 = xs[:, :, 1:57:2]   # x[2t+1]
            e2 = xs[:, :, 2:58:2]   # x[2t+2]
            e3 = xs[:, :, 3:58:2]   # x[2t+3]
            nc.vector.tensor_tensor(dtile[:, 0], e0, e2, sub)
            nc.vector.tensor_tensor(dtile[:, 1], e1, e2, add)
            nc.vector.tensor_tensor(dtile[:, 2], e2, e1, sub)
            nc.vector.tensor_tensor(dtile[:, 3], e1, e3, sub)
            dts[cih] = dtile

        for coh in range(2):
            for grp in range(NG):
                r0 = grp * RG
                m = [pspool.tile([128, FREE_W], dt.float32, tag=f"m{k}",
                                 name=f"m{k}") for k in range(4)]
                for k in range(4):
                    i_mm = 0
                    for ky in range(3):
                        for cih in range(2):
                            lhsT = t_wg[:, cih, k, ky, coh, :]
                            rhs = dts[cih][:, k, r0 + ky: r0 + ky + RG, :]
                            nc.tensor.matmul(m[k][:], lhsT, rhs,
                                             start=(i_mm == 0),
                                             stop=(i_mm == 5))
                            i_mm += 1
                # inverse transform: o_even = m0 + (m1 + m2),
                # o_odd = (m1 - m2) - m3   (1/2 factors folded into G1, G2).
                # DVE can read only ONE PSUM operand per op, so m1 goes to
                # SBUF first (on GpSimd, which is otherwise idle).
                c1 = tpool.tile([128, FREE_W], dt.float32, tag="c1")
                nc.scalar.activation(
                    c1[:], m[1][:], mybir.ActivationFunctionType.Identity,
                    scale=1.0)
                t_s = tpool.tile([128, FREE_W], dt.float32, tag="t_s")
                t_d = tpool.tile([128, FREE_W], dt.float32, tag="t_d")
                nc.vector.tensor_tensor(t_s[:], c1[:], m[2][:], add)
                nc.vector.tensor_tensor(t_d[:], c1[:], m[2][:], sub)
                ob = opool.tile([128, RG, W], dt.float32, tag="ob")
                ov = ob[:].rearrange("p r (t two) -> p r t two", two=2)
                nc.vector.tensor_tensor(
                    ov[:, :, :, 0], m[0][:].rearrange("p (r t) -> p r t", t=T),
                    t_s[:].rearrange("p (r t) -> p r t", t=T), add)
                nc.vector.tensor_tensor(
                    ov[:, :, :, 1], t_d[:].rearrange("p (r t) -> p r t", t=T),
                    m[3][:].rearrange("p (r t) -> p r t", t=T), sub)
                of = opool.tile([128, RG, W], dt.float32, tag="of")
                nc.scalar.activation(
                    of[:].rearrange("p r c -> p (r c)"),
                    ob[:].rearrange("p r c -> p (r c)"),
                    mybir.ActivationFunctionType.Identity,
                    bias=t_bias[:, coh:coh + 1], scale=float(scale))
                nc.scalar.dma_start(
                    d_y[img, coh * 128:(coh + 1) * 128, r0:r0 + RG, :],
                    of[:])


# ──────────────────────────── direct modes ────────────────────────────


def _build_direct(scale: float, mode: str, reps: int, unroll: int):
    nc = bacc.Bacc()
    dt = mybir.dt
    x_dt = dt.float32r if mode == "f32r" else dt.bfloat16
    w_dt = x_dt
    if mode in ("f32r", "bf16"):
        d_xs = [nc.declare_dram_parameter(
            "x0", [IMGS_PER_CORE, C, HP, WP], x_dt, isOutput=False)]
        parts = ("x0",)
    else:
        d_xs = [
            nc.declare_dram_parameter(
                "xhi", [IMGS_PER_CORE, C, HP, WP], x_dt, isOutput=False),
            nc.declare_dram_parameter(
                "xlo", [IMGS_PER_CORE, C, HP, WP], x_dt, isOutput=False),
        ]
        parts = ("xhi", "xlo")
    # wq[ci, ci_half, tap, co_half, co] = (w_q - zp)[co_half*128+co, ci_half*128+ci, tap]
    d_wq = nc.declare_dram_parameter("wq", [128, 2, 9, 2, 128], w_dt, isOutput=False)
    d_bias = nc.declare_dram_parameter("bias", [128, 2], dt.float32, isOutput=False)
    d_y = nc.declare_dram_parameter(
        "y", [IMGS_PER_CORE, C, H, W], dt.float32, isOutput=True)

    with tile.TileContext(nc) as tc:
        with (
            tc.tile_pool(name="wpool", bufs=1) as wpool,
            tc.tile_pool(name="xpool", bufs=3) as xpool,
            tc.tile_pool(name="opool", bufs=4) as opool,
            tc.tile_pool(name="pspool", bufs=4, space="PSUM") as pspool,
        ):
            t_wq = wpool.tile([128, 2, 9, 2, 128], w_dt, tag="wq")
            nc.sync.dma_start(t_wq[:], d_wq[:])
            t_bias = wpool.tile([128, 2], dt.float32, tag="bias")
            nc.sync.dma_start(t_bias[:], d_bias[:])

            loop_cm = (tc.For_i(0, reps) if (reps > 1 or unroll > 1)
                       else contextlib.nullcontext())
            with loop_cm:
                for _u in range(unroll):
                    _emit_direct_body(nc, xpool, opool, pspool, t_wq, t_bias,
                                      d_xs, d_y, parts, x_dt, scale)

    nc.compile()
    return nc


def _emit_direct_body(nc, xpool, opool, pspool, t_wq, t_bias, d_xs, d_y,
                      parts, x_dt, scale):
    dt = mybir.dt
    n_mm = 18 * len(parts)
    for img in range(IMGS_PER_CORE):
        # x tiles for this image: [128 ci, 58, 58] per (half, part)
        xt = {}
        for ci_half in range(2):
            for part, src in zip(parts, d_xs):
                t = xpool.tile([128, HP, WP], x_dt,
                               tag=f"x_{part}{ci_half}")
                nc.sync.dma_start(
                    t[:], src[img, ci_half * 128:(ci_half + 1) * 128])
                xt[(ci_half, part)] = t

        for co_half in range(2):
            for blk in range(N_BLOCKS):
                r0 = blk * R
                ps = pspool.tile([128, FREE], dt.float32, tag="ps")
                i_mm = 0
                for ky in (0, -1, 1):
                    for kx in (-1, 0, 1):
                        tap = (ky + 1) * 3 + (kx + 1)
                        for ci_half in range(2):
                            lhsT = t_wq[:, ci_half, tap, co_half, :]
                            for part in parts:
                                rhs = xt[(ci_half, part)][
                                    :, r0 + ky + 1: r0 + ky + 1 + R,
                                    kx + 1: kx + 1 + W]
                                nc.tensor.matmul(
                                    ps[:], lhsT, rhs,
                                    start=(i_mm == 0),
                                    stop=(i_mm == n_mm - 1))
                                i_mm += 1
                ob = opool.tile([128, FREE], dt.float32, tag="ob")
                nc.scalar.activation(
                    ob[:], ps[:], mybir.ActivationFunctionType.Identity,
                    bias=t_bias[:, co_half:co_half + 1], scale=float(scale))
                nc.scalar.dma_start(
                    d_y[img, co_half * 128:(co_half + 1) * 128,
                        r0:r0 + R, :],
                    ob[:].rearrange("p (r c) -> p r c", c=W))


# ──────────────────────────── host side ────────────────────────────


def _quantize_weight(weight: np.ndarray):
    """Bit-exact fp32 replica of the reference affine-uint8 quantization.
    Returns (w_int, scale): w_int = w_q - zero_point (integers in [-255, 255],
    exact in bf16) and the per-tensor fp32 scale."""
    w = np.asarray(weight, dtype=np.float32)
    min_val = np.min(w)
    max_val = np.max(w)
    scale = np.float32(np.float32(max_val - min_val) / np.float32(255.0))
    zp = np.round(np.clip(np.float32(255.0) - np.float32(max_val / scale),
                          np.float32(0.0), np.float32(255.0)))
    w_q = np.round(np.clip(zp + w / scale, np.float32(0.0), np.float32(255.0)))
    w_int = (w_q - zp).astype(np.float32)
    return w_int, scale


def _make_in_maps(x, weight, bias):
    x = np.asarray(x, dtype=np.float32)
    weight = np.asarray(weight, dtype=np.float32)
    bias = np.asarray(bias, dtype=np.float32)

    w_int, scale = _quantize_weight(weight)
    bias_host = np.ascontiguousarray(bias.reshape(2, 128).T)  # [128, 2]

    # pad to [N, C, 58, 58]
    xp = np.zeros((N_IMGS, C, HP, WP), dtype=np.float32)
    xp[:, :, 1:1 + H, 1:1 + W] = x

    if MODE == "wino":
        # G transform per kernel row, 1/2 of the inverse transform folded in
        w0, w1, w2 = w_int[..., 0], w_int[..., 1], w_int[..., 2]  # [co,ci,ky]
        G = np.stack([w0, (w0 + w1 + w2) / 2, (w0 - w1 + w2) / 2, w2],
                     axis=0)  # [4, co, ci, ky]
        # -> [ci, ci_half, k, ky, co_half, co]
        Gr = G.reshape(4, 2, 128, 2, 128, 3)  # [k, coh, co, cih, ci, ky]
        wg_host = np.ascontiguousarray(
            np.transpose(Gr, (4, 3, 0, 5, 1, 2))).astype(_BF16)
        x_parts = {"x0": xp.astype(_BF16)}
        extra = {"wg": wg_host}
    else:
        # lhsT layout [ci, ci_half, tap, co_half, co]
        w_r = w_int.reshape(2, 128, 2, 128, 9)  # [coh, co, cih, ci, tap]
        wq_host = np.ascontiguousarray(np.transpose(w_r, (3, 2, 4, 0, 1)))
        if MODE == "f32r":
            x_parts = {"x0": xp}  # raw f32 bits; PE rounds on ingest
            extra = {"wq": wq_host.astype(np.float32)}
        elif MODE == "bf16":
            # single bf16 part: w_int is exact in bf16; only x rounds,
            # giving ~1.5e-3 relative output error
            x_parts = {"x0": xp.astype(_BF16)}
            extra = {"wq": wq_host.astype(_BF16)}
        else:
            x_hi = xp.astype(_BF16)
            x_lo = (xp - x_hi.astype(np.float32)).astype(_BF16)
            x_parts = {"xhi": x_hi, "xlo": x_lo}
            extra = {"wq": wq_host.astype(_BF16)}

    in_maps = []
    for c in range(N_CORES):
        sl = slice(c * IMGS_PER_CORE, (c + 1) * IMGS_PER_CORE)
        m = {name: arr[sl] for name, arr in x_parts.items()}
        m.update(extra)
        m["bias"] = bias_host
        in_maps.append(m)
    return in_maps


def kernel(x, weight, bias):
    weight = np.asarray(weight, dtype=np.float32)
    _, scale = _quantize_weight(weight)
    in_maps = _make_in_maps(x, weight, bias)
    nc = _build(float(scale), MODE)
    res = run_bass_kernel_spmd(nc, in_maps, list(range(N_CORES)))
    return np.concatenate([res.results[c]["y"] for c in range(N_CORES)], axis=0)
